# revision 8
# baseline (speedup 1.0000x reference)
"""Multi-head self-attention (B=4, N=1024, DIM=1024, H=16, DH=64) on 8 TRN2
NeuronCores.

Sharding: core c handles batch b = c//2, query rows s*512..(s+1)*512 with
s = c%2 (sequence split). Each core computes K/V for its whole batch
(duplicated across the pair) so no collectives are needed; outputs are
disjoint row slices of `out` and `attn`.

Per-core dataflow (all matmuls bf16 inputs, f32 PSUM accumulation):
  QT[d',i]  = Wq^T X^T   (lhsT=Wq[k,d'], rhs=X^T[k,i])
  KT[d',j]  = Wk^T X^T
  V[j,d']   = X Wv       (lhsT=X^T[k,j], rhs=Wv[k,d'])
  per head h:
    S^T[j,i] = K_h Q_h^T  -> PT = exp(SCALE*S^T)      (for the PV matmul)
    S[i,j]   = Q_h K_h^T  -> exp with fused row-sum   (for the attn output)
    attn     = exp(S)/rowsum -> DRAM
    O[i,d]   = PT^T V_h (lhsT=PT[j,i], rhs=V_h[j,d]) scaled by 1/rowsum[i]
    O^T via PE transpose -> O_sb[d',i]
  Y[i,m] = O_sb^T Wo + bo  (bias via a K=1 matmul with a ones row)
"""

import os
import sys
import threading

import numpy as np

sys.path.insert(0, "/opt/trn_rl_repo")

import ml_dtypes

B, N, DIM, H, DH = 4, 1024, 1024, 16, 64
SCALE = DH**-0.5
P = 128
NCORES = 8
ROWS = N // 2          # query rows per core
KB = DIM // P          # contraction sub-tiles
DB = (H * DH) // P     # d' blocks
NJB = N // P           # key blocks
NIB = ROWS // P        # query-row blocks
F = 512                # matmul free dim / PSUM bank

_lock = threading.Lock()
_runner = None


def build_bass():
    import concourse.bacc as bacc
    import concourse.mybir as mybir
    import concourse.tile as tile
    from concourse.masks import make_identity

    f32 = mybir.dt.float32
    bf16 = mybir.dt.bfloat16
    EXP = mybir.ActivationFunctionType.Exp

    # Bacc (not raw Bass): its compile() runs generate_event_semaphores,
    # which splits multi-wait instructions into single-wait event sems —
    # required by this walrus build.
    nc = bacc.Bacc(
        "TRN2", target_bir_lowering=False, debug=False, num_devices=NCORES
    )
    xt_e = nc.declare_dram_parameter("xt", [DIM, N], bf16, isOutput=False)
    xq_e = nc.declare_dram_parameter("xq", [DIM, ROWS], bf16, isOutput=False)
    wq_e = nc.declare_dram_parameter("wq", [DIM, H * DH], bf16, isOutput=False)
    wk_e = nc.declare_dram_parameter("wk", [DIM, H * DH], bf16, isOutput=False)
    wv_e = nc.declare_dram_parameter("wv", [DIM, H * DH], bf16, isOutput=False)
    wo_e = nc.declare_dram_parameter("wo", [H * DH, DIM], bf16, isOutput=False)
    bo_e = nc.declare_dram_parameter("bo", [DIM], f32, isOutput=False)
    out_e = nc.declare_dram_parameter("out", [ROWS, DIM], f32, isOutput=True)
    attn_e = nc.declare_dram_parameter("attn", [H, ROWS, N], f32, isOutput=True)

    with tile.TileContext(nc) as tc:
        with (
            tc.tile_pool(name="const", bufs=1) as const,
            tc.tile_pool(name="pt", bufs=2) as ptp,
            tc.tile_pool(name="attnw", bufs=3) as attnw,
            tc.tile_pool(name="otmp", bufs=3) as otmp,
            tc.tile_pool(name="small", bufs=12) as small,
            tc.tile_pool(name="yout", bufs=3) as youtp,
            tc.tile_pool(name="ps", bufs=4, space="PSUM") as psp,
            tc.tile_pool(name="pss", bufs=2, space="PSUM") as pssp,
        ):
            # ---- stage inputs ----
            xt_sb = const.tile([P, KB, N], bf16)
            nc.sync.dma_start(xt_sb[:], xt_e.rearrange("(o p) n -> p o n", p=P))
            xq_sb = const.tile([P, KB, ROWS], bf16)
            nc.sync.dma_start(xq_sb[:], xq_e.rearrange("(o p) n -> p o n", p=P))
            wq_sb = const.tile([P, KB, H * DH], bf16)
            nc.sync.dma_start(wq_sb[:], wq_e.rearrange("(o p) n -> p o n", p=P))
            wk_sb = const.tile([P, KB, H * DH], bf16)
            nc.sync.dma_start(wk_sb[:], wk_e.rearrange("(o p) n -> p o n", p=P))
            wv_sb = const.tile([P, KB, H * DH], bf16)
            nc.sync.dma_start(wv_sb[:], wv_e.rearrange("(o p) n -> p o n", p=P))
            wo_sb = const.tile([P, DB, DIM], bf16)
            nc.sync.dma_start(wo_sb[:], wo_e.rearrange("(o p) n -> p o n", p=P))
            bo_f = const.tile([1, DIM], f32)
            nc.sync.dma_start(bo_f[:1, :], bo_e[None, :])
            bo_bf = const.tile([1, DIM], bf16)
            nc.vector.tensor_copy(bo_bf[:1, :], bo_f[:1, :])
            ones_bf = const.tile([1, P], bf16)
            nc.vector.memset(ones_bf[:1, :], 1.0)
            ident = const.tile([P, P], bf16)
            make_identity(nc, ident[:])

            qt_sb = const.tile([P, DB, ROWS], bf16)
            kt_sb = const.tile([P, DB, N], bf16)
            v_sb = const.tile([P, NJB, H * DH], bf16)
            o_sb = const.tile([P, DB, ROWS], bf16)

            # ---- projections ----
            # QT[d'块, i] : lhsT=Wq[k, d'], rhs=X^T[k, i]
            for db in range(DB):
                ps = psp.tile([P, F], mybir.dt.float32, tag="ps")
                for kb in range(KB):
                    nc.tensor.matmul(
                        ps[:],
                        wq_sb[:, kb, db * P : (db + 1) * P],
                        xq_sb[:, kb, :],
                        start=(kb == 0),
                        stop=(kb == KB - 1),
                    )
                nc.vector.tensor_copy(qt_sb[:, db, :], ps[:])
            # KT[d'块, j]
            for db in range(DB):
                for jc in range(N // F):
                    ps = psp.tile([P, F], mybir.dt.float32, tag="ps")
                    for kb in range(KB):
                        nc.tensor.matmul(
                            ps[:],
                            wk_sb[:, kb, db * P : (db + 1) * P],
                            xt_sb[:, kb, jc * F : (jc + 1) * F],
                            start=(kb == 0),
                            stop=(kb == KB - 1),
                        )
                    nc.vector.tensor_copy(kt_sb[:, db, jc * F : (jc + 1) * F], ps[:])
            # V[j块, d'] : lhsT=X^T[k, j], rhs=Wv[k, d']
            for jb in range(NJB):
                for dc in range((H * DH) // F):
                    ps = psp.tile([P, F], mybir.dt.float32, tag="ps")
                    for kb in range(KB):
                        nc.tensor.matmul(
                            ps[:],
                            xt_sb[:, kb, jb * P : (jb + 1) * P],
                            wv_sb[:, kb, dc * F : (dc + 1) * F],
                            start=(kb == 0),
                            stop=(kb == KB - 1),
                        )
                    nc.vector.tensor_copy(v_sb[:, jb, dc * F : (dc + 1) * F], ps[:])

            # ---- attention, one head at a time ----
            for h in range(H):
                hb, ho = h // 2, (h % 2) * DH
                qt_h = qt_sb[ho : ho + DH, hb, :]   # [64, 512]
                kt_h = kt_sb[ho : ho + DH, hb, :]   # [64, 1024]

                # S^T[j, i] -> PT = exp(SCALE * S^T), bf16
                pt = ptp.tile([P, NJB, ROWS], bf16, tag="pt")
                for jb in range(NJB):
                    ps = psp.tile([P, F], mybir.dt.float32, tag="ps")
                    nc.tensor.matmul(
                        ps[:],
                        kt_h[:, jb * P : (jb + 1) * P],
                        qt_h[:, :],
                        start=True,
                        stop=True,
                    )
                    nc.scalar.activation(pt[:, jb, :], ps[:], EXP, scale=SCALE)

                recips = []
                for ib in range(NIB):
                    # S[i, j] in two 512-chunks; exp with fused row-sums
                    at = attnw.tile([P, N], mybir.dt.float32, tag="attnw")
                    rs0 = small.tile([P, 1], mybir.dt.float32, tag="rs0")
                    rs1 = small.tile([P, 1], mybir.dt.float32, tag="rs1")
                    for jc in range(N // F):
                        ps = psp.tile([P, F], mybir.dt.float32, tag="ps")
                        nc.tensor.matmul(
                            ps[:],
                            qt_h[:, ib * P : (ib + 1) * P],
                            kt_h[:, jc * F : (jc + 1) * F],
                            start=True,
                            stop=True,
                        )
                        nc.scalar.activation(
                            at[:, jc * F : (jc + 1) * F],
                            ps[:],
                            EXP,
                            scale=SCALE,
                            accum_out=(rs0 if jc == 0 else rs1)[:, :],
                        )
                    rst = small.tile([P, 1], mybir.dt.float32, tag="rst")
                    nc.vector.tensor_add(rst[:], rs0[:], rs1[:])
                    recip = small.tile([P, 1], mybir.dt.float32, tag="recip")
                    nc.vector.reciprocal(recip[:], rst[:])
                    recips.append(recip)
                    nc.vector.tensor_scalar_mul(at[:], at[:], recip[:])
                    nc.sync.dma_start(attn_e[h, ib * P : (ib + 1) * P, :], at[:])

                # O[i, d] = sum_j PT[j, i]^T V_h[j, d], then scale rows by recip
                for ib in range(NIB):
                    pso = pssp.tile([P, P], mybir.dt.float32, tag="pss")
                    for jb in range(NJB):
                        nc.tensor.matmul(
                            pso[:, :DH],
                            pt[:, jb, ib * P : (ib + 1) * P],
                            v_sb[:, jb, h * DH : (h + 1) * DH],
                            start=(jb == 0),
                            stop=(jb == NJB - 1),
                        )
                    ot = otmp.tile([P, DH], bf16, tag="otmp")
                    nc.vector.tensor_scalar_mul(ot[:], pso[:, :DH], recips[ib][:])
                    # transpose [128 i, 64 d] -> [64 d, 128 i]
                    pst = pssp.tile([P, P], bf16, tag="psst")
                    nc.tensor.transpose(pst[:DH, :], ot[:], ident[:])
                    nc.vector.tensor_copy(
                        o_sb[ho : ho + DH, hb, ib * P : (ib + 1) * P], pst[:DH, :]
                    )

            # ---- output projection: Y[i, m] = O_sb^T Wo + bo ----
            for ib in range(NIB):
                for mc in range(DIM // F):
                    ps = psp.tile([P, F], mybir.dt.float32, tag="ps")
                    for db in range(DB):
                        nc.tensor.matmul(
                            ps[:],
                            o_sb[:, db, ib * P : (ib + 1) * P],
                            wo_sb[:, db, mc * F : (mc + 1) * F],
                            start=(db == 0),
                            stop=False,
                        )
                    nc.tensor.matmul(
                        ps[:],
                        ones_bf[:1, :],
                        bo_bf[:1, mc * F : (mc + 1) * F],
                        start=False,
                        stop=True,
                    )
                    yt = youtp.tile([P, F], mybir.dt.float32, tag="yt")
                    nc.vector.tensor_copy(yt[:], ps[:])
                    nc.sync.dma_start(
                        out_e[ib * P : (ib + 1) * P, mc * F : (mc + 1) * F], yt[:]
                    )

    nc.compile()
    return nc


class _Runner:
    """Builds the Bass graph once and a reusable sharded PJRT callable."""

    def __init__(self):
        import jax
        import jax.numpy as jnp  # noqa: F401
        from jax.sharding import Mesh, PartitionSpec
        from jax.experimental.shard_map import shard_map
        import concourse.mybir as mybir
        from concourse import bass2jax

        self.nc = build_bass()
        bass2jax.install_neuronx_cc_hook()

        in_names: list[str] = []
        out_names: list[str] = []
        out_avals = []
        zero_shapes = []
        partition_name = (
            self.nc.partition_id_tensor.name if self.nc.partition_id_tensor else None
        )
        for alloc in self.nc.m.functions[0].allocations:
            if not isinstance(alloc, mybir.MemoryLocationSet):
                continue
            name = alloc.memorylocations[0].name
            if alloc.kind == "ExternalInput":
                if name != partition_name:
                    in_names.append(name)
            elif alloc.kind == "ExternalOutput":
                out_names.append(name)
                shape = tuple(alloc.tensor_shape)
                dtype = mybir.dt.np(alloc.dtype)
                out_avals.append(jax.core.ShapedArray(shape, dtype))
                zero_shapes.append((shape, dtype))
        self.in_names = in_names
        self.out_names = out_names
        self.out_avals = out_avals
        self.zero_shapes = zero_shapes
        n_params = len(in_names)
        n_outs = len(out_names)
        all_in_names = list(in_names) + list(out_names)
        if partition_name is not None:
            all_in_names.append(partition_name)

        nc = self.nc

        def _body(*args):
            operands = list(args)
            if partition_name is not None:
                operands.append(bass2jax.partition_id_tensor())
            outs = bass2jax._bass_exec_p.bind(
                *operands,
                out_avals=tuple(out_avals),
                in_names=tuple(all_in_names),
                out_names=tuple(out_names),
                lowering_input_output_aliases=(),
                sim_require_finite=True,
                sim_require_nnan=True,
                nc=nc,
            )
            return tuple(outs)

        devices = jax.devices()[:NCORES]
        assert len(devices) == NCORES
        self.mesh = Mesh(np.asarray(devices), ("core",))
        in_specs = (PartitionSpec("core"),) * (n_params + n_outs)
        out_specs = (PartitionSpec("core"),) * n_outs
        self.sharded = jax.jit(
            shard_map(
                _body,
                mesh=self.mesh,
                in_specs=in_specs,
                out_specs=out_specs,
                check_rep=False,
            ),
            donate_argnums=tuple(range(n_params, n_params + n_outs)),
            keep_unused=True,
        )

    def concat_inputs(self, in_maps):
        return [
            np.concatenate([np.asarray(m[name]) for m in in_maps], axis=0)
            for name in self.in_names
        ]

    def zeros(self):
        return [
            np.zeros((NCORES * s[0], *s[1:]), d) for (s, d) in self.zero_shapes
        ]

    def run(self, in_maps):
        out_arrs = self.sharded(*self.concat_inputs(in_maps), *self.zeros())
        res = []
        for c in range(NCORES):
            res.append(
                {
                    name: np.asarray(out_arrs[i]).reshape(
                        NCORES, *self.out_avals[i].shape
                    )[c]
                    for i, name in enumerate(self.out_names)
                }
            )
        return res


def _get_runner():
    global _runner
    with _lock:
        if _runner is None:
            _runner = _Runner()
    return _runner


def make_in_maps(x, Wq, Wk, Wv, Wo, bo):
    bf16 = ml_dtypes.bfloat16
    xt = np.ascontiguousarray(np.transpose(np.asarray(x, np.float32), (0, 2, 1))).astype(bf16)
    wq = np.asarray(Wq, np.float32).astype(bf16)
    wk = np.asarray(Wk, np.float32).astype(bf16)
    wv = np.asarray(Wv, np.float32).astype(bf16)
    wo = np.asarray(Wo, np.float32).astype(bf16)
    bo = np.ascontiguousarray(np.asarray(bo, np.float32))
    in_maps = []
    for c in range(NCORES):
        b, s = c // 2, c % 2
        in_maps.append(
            {
                "xt": xt[b],
                "xq": np.ascontiguousarray(xt[b][:, s * ROWS : (s + 1) * ROWS]),
                "wq": wq,
                "wk": wk,
                "wv": wv,
                "wo": wo,
                "bo": bo,
            }
        )
    return in_maps


def gather_outputs(results):
    out = np.empty((B, N, DIM), np.float32)
    attn = np.empty((B, H, N, N), np.float32)
    for c, r in enumerate(results):
        b, s = c // 2, c % 2
        out[b, s * ROWS : (s + 1) * ROWS, :] = r["out"]
        attn[b, :, s * ROWS : (s + 1) * ROWS, :] = r["attn"]
    return out, attn


def kernel(x, Wq, Wk, Wv, Wo, bo):
    runner = _get_runner()
    results = runner.run(make_in_maps(x, Wq, Wk, Wv, Wo, bo))
    return gather_outputs(results)


def bench(x, Wq, Wk, Wv, Wo, bo, iters=8):
    """Estimate per-NEFF device time by pipelining `iters` executions with
    all operands staged on device ahead of the timed loop."""
    import time
    import jax
    from jax.sharding import NamedSharding, PartitionSpec

    runner = _get_runner()
    sh = NamedSharding(runner.mesh, PartitionSpec("core"))
    cin = [
        jax.device_put(a, sh)
        for a in runner.concat_inputs(make_in_maps(x, Wq, Wk, Wv, Wo, bo))
    ]
    # Warm-up (also triggers compile)
    res = runner.sharded(*cin, *[jax.device_put(z, sh) for z in runner.zeros()])
    jax.block_until_ready(res)

    zsets = [
        [jax.device_put(z, sh) for z in runner.zeros()] for _ in range(iters)
    ]
    jax.block_until_ready(zsets)
    t0 = time.perf_counter()
    outs = [runner.sharded(*cin, *z) for z in zsets]
    jax.block_until_ready(outs)
    t1 = time.perf_counter()
    total = t1 - t0

    # single-call wall time for overhead reference
    z1 = [jax.device_put(z, sh) for z in runner.zeros()]
    jax.block_until_ready(z1)
    t2 = time.perf_counter()
    o1 = runner.sharded(*cin, *z1)
    jax.block_until_ready(o1)
    t3 = time.perf_counter()
    single = t3 - t2

    per_iter_ns = total / iters * 1e9
    return {
        "per_iter_ns": per_iter_ns,
        "single_call_ns": single * 1e9,
        "total_s": total,
    }
